# revision 15
# baseline (speedup 1.0000x reference)
"""MoE top-1 routing kernel for Trainium2 (8 NeuronCores).

Math (matches the reference):
    logits = x @ gate_w + gate_b            # [N, E]
    assign = argmax(logits, -1)             # top-1 expert per token
    out[t] = relu(x[t] @ w1[e] + b1[e]) @ w2[e] + b2[e]   where e = assign[t]

The gate is a tiny (4096x1024x8) matmul computed on the host in float64 (the
smallest top1-top2 logit gap in this regime is ~2e-4, orders of magnitude
above fp32 rounding, so the argmax is unambiguous). Tokens are grouped by
expert and dispatched to the cores holding that expert's weights; outputs are
scattered back to token order on the host.

Device sharding: 2-way tensor-parallel expert pairs. Experts are paired
large-count-with-small-count; the two cores of a pair each hold HALF of the
DFF dimension of BOTH experts and process all the pair's tokens through their
DFF half. relu is elementwise so layer 1 halves are independent; layer 2
produces partial sums over the DFF half which the host adds.

The matmul path is bf16 (inputs quantized on the host, PSUM accumulation is
fp32): same 1 column/cycle PE rate as float32r but half the HBM traffic, and
LDWEIGHTS gets the fast-weight-load path (2 bf16/cycle) so the weight loads
hide under the matmul stream. End-to-end quantization error is ~5e-4 absmax
against the fp32 reference, far inside the 2e-2 gate.

Per-core loop structure: contraction (k) outer, token-chunk inner, so each
128x128 weight block is loaded once and feeds every token chunk back-to-back
(two PSUM banks carry the per-chunk accumulations). A short run of dummy
matmuls on a zeroed tile warms the PE clock (HAM throttle) during the initial
weight/token DMA, and the first real matmul only waits for a 32KB weight
slice plus one xt k-slice.
"""

import numpy as np

N_TOK, D, DFF, E = 4096, 1024, 4096, 8
P = 128
KD = D // P  # 8 contraction chunks of the d dimension
MH = (DFF // 2) // P  # 16 dff-half blocks (layer1 out / layer2 contraction)

NWARM = 42  # PE-clock warmup matmuls issued while the first DMAs land

# test.py hooks: set TRACE=True (after installing the NTFF hook) to profile.
TRACE = False
TRACE_CORES = None
LAST_RESULT = None

_PROGRAM_CACHE = {}


def _bf16(a):
    import ml_dtypes

    return np.ascontiguousarray(np.asarray(a).astype(ml_dtypes.bfloat16))


def _pad_cap(n):
    """Token capacity: multiple of 8 (DMA alignment), floor 64."""
    return max(64, -(-n // 8) * 8)


def _chunk_sizes(C):
    """Split C tokens into moving-dim chunks <=512 (PSUM bank), balanced."""
    n = -(-C // 512)
    base, rem = divmod(C, n)
    return [base + (1 if i < rem else 0) for i in range(n)]


def _build_program_tp2(C1, C2):
    import concourse.mybir as mybir
    import concourse.tile as tile
    from concourse import bacc

    f32 = mybir.dt.float32
    bf16 = mybir.dt.bfloat16
    AF = mybir.ActivationFunctionType

    chunks1 = _chunk_sizes(C1)
    chunks2 = _chunk_sizes(C2)

    nc = bacc.Bacc("TRN2", target_bir_lowering=False, debug=False, num_devices=E)

    xt1_d = nc.dram_tensor("xt1", [P, KD * C1], bf16, kind="ExternalInput").ap()
    xt2_d = nc.dram_tensor("xt2", [P, KD * C2], bf16, kind="ExternalInput").ap()
    w1a_d = nc.dram_tensor("w1a", [MH, P, D], bf16, kind="ExternalInput").ap()
    w1b_d = nc.dram_tensor("w1b", [MH, P, D], bf16, kind="ExternalInput").ap()
    b1a_d = nc.dram_tensor("b1a", [P, MH], f32, kind="ExternalInput").ap()
    b1b_d = nc.dram_tensor("b1b", [P, MH], f32, kind="ExternalInput").ap()
    w2a_d = nc.dram_tensor("w2a", [KD, P, MH * P], bf16, kind="ExternalInput").ap()
    w2b_d = nc.dram_tensor("w2b", [KD, P, MH * P], bf16, kind="ExternalInput").ap()
    yt1_d = nc.dram_tensor("yt1", [KD, P, C1], bf16, kind="ExternalOutput").ap()
    yt2_d = nc.dram_tensor("yt2", [KD, P, C2], bf16, kind="ExternalOutput").ap()

    c0 = chunks1[0]
    # chunk-major xt layout: [P, chunk0(k-major), chunk1(k-major), ...] so any
    # k-range of a chunk is a fully contiguous (wide-line) DMA
    off1 = [0]
    for tn in chunks1:
        off1.append(off1[-1] + KD * tn)
    off2 = [0]
    for tn in chunks2:
        off2.append(off2[-1] + KD * tn)

    with tile.TileContext(nc) as tc:
        with (
            tc.tile_pool(name="xt_pool", bufs=1) as xt_pool,
            tc.tile_pool(name="ht_pool", bufs=1) as ht_pool,
            tc.tile_pool(name="w1_pool", bufs=16) as w1_pool,
            tc.tile_pool(name="w2_pool", bufs=12) as w2_pool,
            tc.tile_pool(name="y_pool", bufs=3) as y_pool,
            tc.tile_pool(name="bias_pool", bufs=1) as bias_pool,
            tc.tile_pool(name="psum", bufs=3, space="PSUM") as psum_pool,
        ):
            # PE clock warmup: dummy matmuls on a zeroed tile keep the PE
            # busy (HAM sees activity -> full 2.4GHz clock) while the first
            # weight/token DMAs are still in flight. Results land in a
            # dedicated PSUM bank and are never read.
            warm_sb = bias_pool.tile([P, 512], bf16)
            nc.vector.memset(warm_sb[:], 0)
            warm_ps = psum_pool.tile([P, 512], f32, tag="warm", bufs=1)
            for _ in range(NWARM):
                nc.tensor.matmul(
                    warm_ps[:], lhsT=warm_sb[:, :P], rhs=warm_sb[:], start=True,
                    stop=True,
                )

            xt1_sb = xt_pool.tile([P, KD * C1], bf16)
            xt2_sb = xt_pool.tile([P, KD * C2], bf16)
            b1a_sb = bias_pool.tile([P, MH], f32)
            b1b_sb = bias_pool.tile([P, MH], f32)

            # Startup loads in data-need order across all three DMA queues.
            # Layer-1a runs in single-chunk passes, so only the chunk-0
            # sub-slices of the xt k-slices (plus the first weight blocks)
            # gate early compute; everything else streams in behind.
            # Startup: the first ~1MB (xt chunk-0 + first weight blocks) is
            # the cold-queue critical path, so spread it round-robin across
            # all three DMA queues in data-need order.
            w1a_sbs = [
                w1_pool.tile([P, D], bf16, tag="w1", name=f"w1a{m}")
                for m in range(MH)
            ]
            # sync HWDGE ramps first and fastest: give it the head of the
            # critical path (xt k0 + first weight blocks, interleaved)
            nc.sync.dma_start(xt1_sb[:, :c0], xt1_d[:, :c0])
            nc.sync.dma_start(w1a_sbs[0][:, : 3 * P], w1a_d[0][:, : 3 * P])
            nc.sync.dma_start(xt1_sb[:, c0 : 4 * c0], xt1_d[:, c0 : 4 * c0])
            nc.sync.dma_start(w1a_sbs[0][:, 3 * P :], w1a_d[0][:, 3 * P :])
            nc.sync.dma_start(w1a_sbs[1][:], w1a_d[1])
            nc.sync.dma_start(w1a_sbs[3][:], w1a_d[3])
            # scalar HWDGE: bias + w1 m=4,6,7
            nc.scalar.dma_start(b1a_sb[:], b1a_d[:])
            nc.scalar.dma_start(w1a_sbs[4][:], w1a_d[4])
            nc.scalar.dma_start(w1a_sbs[6][:], w1a_d[6])
            nc.scalar.dma_start(w1a_sbs[7][:], w1a_d[7])
            # gpsimd SWDGE: xt k4-7, w1 m=2,5, then the chunk-1 xt block
            nc.gpsimd.dma_start(xt1_sb[:, 4 * c0 : 8 * c0], xt1_d[:, 4 * c0 : 8 * c0])
            nc.gpsimd.dma_start(w1a_sbs[2][:], w1a_d[2])
            nc.gpsimd.dma_start(w1a_sbs[5][:], w1a_d[5])
            if len(chunks1) > 1:
                nc.gpsimd.dma_start(xt1_sb[:, off1[1] :], xt1_d[:, off1[1] :])
            # rest of the w1a stream on sync
            for m in range(8, MH):
                nc.sync.dma_start(w1a_sbs[m][:], w1a_d[m])

            ht1_sb = ht_pool.tile([P, MH * C1], bf16)
            ht2_sb = ht_pool.tile([P, MH * C2], bf16)

            def l1_block(m, ci, off, t0, tn, w1_sb, C, xt_sb, ht_sb, b1_sb):
                ps = psum_pool.tile([P, 512], f32, tag=f"ps{ci}", name=f"ps{ci}")
                for k in range(KD):
                    nc.tensor.matmul(
                        ps[:, :tn],
                        lhsT=w1_sb[:, k * P : (k + 1) * P],
                        rhs=xt_sb[:, off + k * tn : off + (k + 1) * tn],
                        start=(k == 0),
                        stop=(k == KD - 1),
                    )
                nc.scalar.activation(
                    ht_sb[:, m * C + t0 : m * C + t0 + tn],
                    ps[:, :tn],
                    AF.Relu,
                    bias=b1_sb[:, m : m + 1],
                )

            # Layer 1, slot a: single-chunk passes (pass A only needs the
            # chunk-0 xt bytes; w1a tiles stay resident for pass B)
            t0 = 0
            for ci, tn in enumerate(chunks1):
                for m in range(MH):
                    l1_block(
                        m, 0, off1[ci], t0, tn, w1a_sbs[m], C1, xt1_sb, ht1_sb,
                        b1a_sb,
                    )
                if ci == 0:
                    # slot-b tokens/bias + first layer-2 weights prefetch on
                    # the now-idle gpsimd queue
                    nc.gpsimd.dma_start(xt2_sb[:], xt2_d[:])
                    nc.gpsimd.dma_start(b1b_sb[:], b1b_d[:])
                t0 += tn
            w2_first = w2_pool.tile([P, MH * P], bf16, tag="w2")
            H2 = MH * P // 2
            nc.gpsimd.dma_start(w2_first[:, :H2], w2b_d[0][:, :H2])
            nc.gpsimd.dma_start(w2_first[:, H2:], w2b_d[0][:, H2:])

            # Layer 1, slot b: paired chunks (k-outer), weights ring through
            # the w1 pool behind the resident w1a tiles
            for m in range(MH):
                w1_sb = w1_pool.tile([P, D], bf16, tag="w1")
                nc.sync.dma_start(w1_sb[:], w1b_d[m])
                pss = [
                    psum_pool.tile([P, 512], f32, tag=f"ps{ci}", name=f"ps{ci}")
                    for ci in range(len(chunks2))
                ]
                for k in range(KD):
                    for ci, tn in enumerate(chunks2):
                        nc.tensor.matmul(
                            pss[ci][:, :tn],
                            lhsT=w1_sb[:, k * P : (k + 1) * P],
                            rhs=xt2_sb[:, off2[ci] + k * tn : off2[ci] + (k + 1) * tn],
                            start=(k == 0),
                            stop=(k == KD - 1),
                        )
                t0 = 0
                for ci, tn in enumerate(chunks2):
                    nc.scalar.activation(
                        ht2_sb[:, m * C2 + t0 : m * C2 + t0 + tn],
                        pss[ci][:, :tn],
                        AF.Relu,
                        bias=b1b_sb[:, m : m + 1],
                    )
                    t0 += tn

            def layer2(m2, w2_sb, C, chunks, ht_sb, yt_d, last=False):
                if last:
                    # final block: single-chunk passes so the earlier chunks'
                    # activations+stores overlap the later chunks' matmuls;
                    # the very last chunk's store is split across the two
                    # idle DMA engines to shorten the drain
                    t0 = 0
                    for ci, tn in enumerate(chunks):
                        ps = psum_pool.tile([P, 512], f32, tag="ps0", name="ps")
                        for k2 in range(MH):
                            nc.tensor.matmul(
                                ps[:, :tn],
                                lhsT=w2_sb[:, k2 * P : (k2 + 1) * P],
                                rhs=ht_sb[:, k2 * C + t0 : t0 + k2 * C + tn],
                                start=(k2 == 0),
                                stop=(k2 == MH - 1),
                            )
                        yt_sb = y_pool.tile([P, 512], bf16, tag="yt")
                        nc.scalar.activation(yt_sb[:, :tn], ps[:, :tn], AF.Identity)
                        if ci == len(chunks) - 1:
                            nc.sync.dma_start(
                                yt_d[m2][:, t0 : t0 + tn], yt_sb[:, :tn]
                            )
                        else:
                            nc.gpsimd.dma_start(
                                yt_d[m2][:, t0 : t0 + tn], yt_sb[:, :tn]
                            )
                        t0 += tn
                    return
                pss = [
                    psum_pool.tile([P, 512], f32, tag=f"ps{ci}", name=f"ps{ci}")
                    for ci in range(len(chunks))
                ]
                for k2 in range(MH):
                    t0 = 0
                    for ci, tn in enumerate(chunks):
                        nc.tensor.matmul(
                            pss[ci][:, :tn],
                            lhsT=w2_sb[:, k2 * P : (k2 + 1) * P],
                            rhs=ht_sb[:, k2 * C + t0 : k2 * C + t0 + tn],
                            start=(k2 == 0),
                            stop=(k2 == MH - 1),
                        )
                        t0 += tn
                t0 = 0
                for ci, tn in enumerate(chunks):
                    yt_sb = y_pool.tile([P, 512], bf16, tag="yt")
                    # partial sum over this core's DFF half; b2 added on host
                    nc.scalar.activation(yt_sb[:, :tn], pss[ci][:, :tn], AF.Identity)
                    nc.gpsimd.dma_start(yt_d[m2][:, t0 : t0 + tn], yt_sb[:, :tn])
                    t0 += tn

            for m2 in range(KD):
                if m2 == 0:
                    w2_sb = w2_first
                else:
                    w2_sb = w2_pool.tile([P, MH * P], bf16, tag="w2")
                    nc.sync.dma_start(w2_sb[:], w2b_d[m2])
                layer2(m2, w2_sb, C2, chunks2, ht2_sb, yt2_d)
            for m2 in range(KD):
                w2_sb = w2_pool.tile([P, MH * P], bf16, tag="w2")
                nc.sync.dma_start(w2_sb[:], w2a_d[m2])
                layer2(m2, w2_sb, C1, chunks1, ht1_sb, yt1_d, last=(m2 == KD - 1))

    nc.compile()
    return nc


def _arrange_tokens(x_e, C):
    """[n, D] tokens -> chunk-major xt: for each token chunk [t0:t0+tn],
    a [P, KD*tn] block with [p, k*tn + c] = x_e[t0+c, k*128 + p]."""
    xe = np.zeros((C, D), np.float32)
    xe[: len(x_e)] = x_e
    blocks = []
    t0 = 0
    for tn in _chunk_sizes(C):
        blk = xe[t0 : t0 + tn].T.reshape(KD, P, tn).transpose(1, 0, 2)
        blocks.append(blk.reshape(P, KD * tn))
        t0 += tn
    return _bf16(np.concatenate(blocks, axis=1))


def _arrange_w1_half(w1_e, h):
    """w1 half: [D, 2048] -> [MH, P, D] with [m, p, k*128+j] = w1[k*128+p, off+m*128+j]."""
    half = w1_e[:, h * (MH * P) : (h + 1) * (MH * P)]
    return _bf16(half.reshape(KD, P, MH, P).transpose(2, 1, 0, 3).reshape(MH, P, D))


def _arrange_w2_half(w2_e, h):
    """w2 half: [2048, D] -> [KD, P, 2048] with [m2, p, k2*128+j] = w2[off+k2*128+p, m2*128+j]."""
    half = w2_e[h * (MH * P) : (h + 1) * (MH * P), :]
    return _bf16(half.reshape(MH, P, KD, P).transpose(2, 1, 0, 3).reshape(KD, P, MH * P))


def kernel(x, gate_w, gate_b, w1, b1, w2, b2):
    global LAST_RESULT

    x = np.ascontiguousarray(np.asarray(x, dtype=np.float32))
    gate_w = np.asarray(gate_w, dtype=np.float32)
    gate_b = np.asarray(gate_b, dtype=np.float32)
    w1 = np.asarray(w1, dtype=np.float32)
    b1 = np.asarray(b1, dtype=np.float32)
    w2 = np.asarray(w2, dtype=np.float32)
    b2 = np.asarray(b2, dtype=np.float32)
    n_tok = x.shape[0]

    # host gate + top-1 routing (fp64: exact argmax, see module docstring)
    logits = x.astype(np.float64) @ gate_w.astype(np.float64) + gate_b.astype(
        np.float64
    )
    assign = np.argmax(logits, axis=-1)
    idx_full = [np.nonzero(assign == e)[0] for e in range(E)]

    # Defensive slabbing: if routing were pathologically imbalanced, process
    # tokens in passes so per-expert capacity stays within SBUF limits. With
    # the benchmark's near-uniform gate this is a single pass.
    slab = 960
    n_pass = max(1, -(-max(len(i) for i in idx_full) // slab))
    out = np.zeros((n_tok, D), np.float32)
    for ps in range(n_pass):
        idx = [i[ps * slab : (ps + 1) * slab] for i in idx_full]
        _run_pass(x, w1, b1, w2, b2, idx, out)
    return out


def _run_pass(x, w1, b1, w2, b2, idx, out):
    from concourse.bass_utils import run_bass_kernel_spmd

    global LAST_RESULT

    counts = np.array([len(i) for i in idx])

    # pair experts large-with-small to balance per-core token load
    order = np.argsort(-counts, kind="stable")
    pairs = [(int(order[p]), int(order[E - 1 - p])) for p in range(E // 2)]
    C1 = _pad_cap(max(counts[a] for a, _ in pairs))
    C2 = _pad_cap(max(counts[b] for _, b in pairs))

    key = (C1, C2)
    if key not in _PROGRAM_CACHE:
        _PROGRAM_CACHE[key] = _build_program_tp2(C1, C2)
    nc = _PROGRAM_CACHE[key]

    in_maps = []
    for c in range(E):
        p, h = divmod(c, 2)
        ea, eb = pairs[p]
        in_maps.append(
            {
                "xt1": _arrange_tokens(x[idx[ea]], C1),
                "xt2": _arrange_tokens(x[idx[eb]], C2),
                "w1a": _arrange_w1_half(w1[ea], h),
                "w1b": _arrange_w1_half(w1[eb], h),
                "b1a": np.ascontiguousarray(
                    b1[ea][h * (MH * P) : (h + 1) * (MH * P)].reshape(MH, P).T
                ),
                "b1b": np.ascontiguousarray(
                    b1[eb][h * (MH * P) : (h + 1) * (MH * P)].reshape(MH, P).T
                ),
                "w2a": _arrange_w2_half(w2[ea], h),
                "w2b": _arrange_w2_half(w2[eb], h),
            }
        )

    res = run_bass_kernel_spmd(
        nc,
        in_maps,
        core_ids=list(range(E)),
        trace=TRACE,
        **({"trace_cores": TRACE_CORES} if TRACE_CORES else {}),
    )
    LAST_RESULT = res

    for p in range(E // 2):
        ea, eb = pairs[p]
        for slot, e in (("yt1", ea), ("yt2", eb)):
            n = len(idx[e])
            if n == 0:
                continue
            # sum the two DFF-half partials, restore [tokens, D], add b2
            yt = res.results[2 * p][slot].astype(np.float32) + res.results[
                2 * p + 1
            ][slot].astype(np.float32)
            ye = yt.transpose(2, 0, 1).reshape(-1, D)
            out[idx[e]] = ye[:n] + b2[e]


# revision 16
# speedup vs baseline: 1.0649x; 1.0649x over previous
"""MoE top-1 routing kernel for Trainium2 (8 NeuronCores).

Math (matches the reference):
    logits = x @ gate_w + gate_b            # [N, E]
    assign = argmax(logits, -1)             # top-1 expert per token
    out[t] = relu(x[t] @ w1[e] + b1[e]) @ w2[e] + b2[e]   where e = assign[t]

The gate is a tiny (4096x1024x8) matmul computed on the host in float64 (the
smallest top1-top2 logit gap in this regime is ~2e-4, orders of magnitude
above fp32 rounding, so the argmax is unambiguous). Tokens are grouped by
expert and dispatched to the cores holding that expert's weights; outputs are
scattered back to token order on the host.

Device sharding: 2-way tensor-parallel expert pairs. Experts are paired
large-count-with-small-count; the two cores of a pair each hold HALF of the
DFF dimension of BOTH experts and process all the pair's tokens through their
DFF half. relu is elementwise so layer 1 halves are independent; layer 2
produces partial sums over the DFF half which the host adds.

The matmul path is bf16 (inputs quantized on the host, PSUM accumulation is
fp32): same 1 column/cycle PE rate as float32r but half the HBM traffic, and
LDWEIGHTS gets the fast-weight-load path (2 bf16/cycle) so the weight loads
hide under the matmul stream. End-to-end quantization error is ~5e-4 absmax
against the fp32 reference, far inside the 2e-2 gate.

Per-core loop structure: contraction (k) outer, token-chunk inner, so each
128x128 weight block is loaded once and feeds every token chunk back-to-back
(two PSUM banks carry the per-chunk accumulations). A short run of dummy
matmuls on a zeroed tile warms the PE clock (HAM throttle) during the initial
weight/token DMA, and the first real matmul only waits for a 32KB weight
slice plus one xt k-slice.
"""

import numpy as np

N_TOK, D, DFF, E = 4096, 1024, 4096, 8
P = 128
KD = D // P  # 8 contraction chunks of the d dimension
MH = (DFF // 2) // P  # 16 dff-half blocks (layer1 out / layer2 contraction)

NWARM = 10  # PE-clock warmup matmuls issued while the first DMAs land

# test.py hooks: set TRACE=True (after installing the NTFF hook) to profile.
TRACE = False
TRACE_CORES = None
LAST_RESULT = None

_PROGRAM_CACHE = {}


def _bf16(a):
    import ml_dtypes

    return np.ascontiguousarray(np.asarray(a).astype(ml_dtypes.bfloat16))


def _pad_cap(n):
    """Token capacity: multiple of 8 (DMA alignment), floor 64."""
    return max(64, -(-n // 8) * 8)


def _chunk_sizes(C):
    """Split C tokens into moving-dim chunks <=512 (PSUM bank), balanced."""
    n = -(-C // 512)
    base, rem = divmod(C, n)
    return [base + (1 if i < rem else 0) for i in range(n)]


def _build_program_tp2(C1, C2):
    import concourse.mybir as mybir
    import concourse.tile as tile
    from concourse import bacc

    f32 = mybir.dt.float32
    bf16 = mybir.dt.bfloat16
    AF = mybir.ActivationFunctionType

    chunks1 = _chunk_sizes(C1)
    chunks2 = _chunk_sizes(C2)

    nc = bacc.Bacc("TRN2", target_bir_lowering=False, debug=False, num_devices=E)

    xt1_d = nc.dram_tensor("xt1", [P, KD * C1], bf16, kind="ExternalInput").ap()
    xt2_d = nc.dram_tensor("xt2", [P, KD * C2], bf16, kind="ExternalInput").ap()
    w1a_d = nc.dram_tensor("w1a", [MH, P, D], bf16, kind="ExternalInput").ap()
    w1b_d = nc.dram_tensor("w1b", [MH, P, D], bf16, kind="ExternalInput").ap()
    b1a_d = nc.dram_tensor("b1a", [P, MH], f32, kind="ExternalInput").ap()
    b1b_d = nc.dram_tensor("b1b", [P, MH], f32, kind="ExternalInput").ap()
    w2a_d = nc.dram_tensor("w2a", [KD, P, MH * P], bf16, kind="ExternalInput").ap()
    w2b_d = nc.dram_tensor("w2b", [KD, P, MH * P], bf16, kind="ExternalInput").ap()
    yt1_d = nc.dram_tensor("yt1", [KD, P, C1], bf16, kind="ExternalOutput").ap()
    yt2_d = nc.dram_tensor("yt2", [KD, P, C2], bf16, kind="ExternalOutput").ap()

    c0 = chunks1[0]
    # chunk-major xt layout: [P, chunk0(k-major), chunk1(k-major), ...] so any
    # k-range of a chunk is a fully contiguous (wide-line) DMA
    off1 = [0]
    for tn in chunks1:
        off1.append(off1[-1] + KD * tn)
    off2 = [0]
    for tn in chunks2:
        off2.append(off2[-1] + KD * tn)

    with tile.TileContext(nc) as tc:
        with (
            tc.tile_pool(name="xt_pool", bufs=1) as xt_pool,
            tc.tile_pool(name="ht_pool", bufs=1) as ht_pool,
            tc.tile_pool(name="w1_pool", bufs=16) as w1_pool,
            tc.tile_pool(name="w2_pool", bufs=12) as w2_pool,
            tc.tile_pool(name="y_pool", bufs=3) as y_pool,
            tc.tile_pool(name="bias_pool", bufs=1) as bias_pool,
            tc.tile_pool(name="psum", bufs=3, space="PSUM") as psum_pool,
        ):
            # PE clock warmup: dummy matmuls on a zeroed tile keep the PE
            # busy (HAM sees activity -> full 2.4GHz clock) while the first
            # weight/token DMAs are still in flight. Results land in a
            # dedicated PSUM bank and are never read.
            warm_sb = bias_pool.tile([P, 512], bf16)
            nc.vector.memset(warm_sb[:], 0)
            warm_ps = psum_pool.tile([P, 512], f32, tag="warm", bufs=1)
            for _ in range(NWARM):
                nc.tensor.matmul(
                    warm_ps[:], lhsT=warm_sb[:, :P], rhs=warm_sb[:], start=True,
                    stop=True,
                )

            xt1_sb = xt_pool.tile([P, KD * C1], bf16)
            xt2_sb = xt_pool.tile([P, KD * C2], bf16)
            b1a_sb = bias_pool.tile([P, MH], f32)
            b1b_sb = bias_pool.tile([P, MH], f32)

            # Startup loads in data-need order across all three DMA queues.
            # Layer-1a runs in single-chunk passes, so only the chunk-0
            # sub-slices of the xt k-slices (plus the first weight blocks)
            # gate early compute; everything else streams in behind.
            # Startup: the first ~1MB (xt chunk-0 + first weight blocks) is
            # the cold-queue critical path, so spread it round-robin across
            # all three DMA queues in data-need order.
            w1a_sbs = [
                w1_pool.tile([P, D], bf16, tag="w1", name=f"w1a{m}")
                for m in range(MH)
            ]
            # Cold-start DMA queues ramp slowly (~30-40GB/s each for the
            # first several us), so the critical tensors (xt chunk-0 and the
            # first weight blocks) are STRIPED across all three queues in
            # data-need order - each stripe is an independent DMA, so the
            # tensor is ready when the slowest third lands.
            nc.sync.dma_start(xt1_sb[:, :c0], xt1_d[:, :c0])
            nc.sync.dma_start(w1a_sbs[0][:, : 2 * P], w1a_d[0][:, : 2 * P])
            nc.scalar.dma_start(xt1_sb[:, c0 : 2 * c0], xt1_d[:, c0 : 2 * c0])
            nc.gpsimd.dma_start(w1a_sbs[0][:, 2 * P : 4 * P], w1a_d[0][:, 2 * P : 4 * P])
            nc.scalar.dma_start(xt1_sb[:, 2 * c0 : 4 * c0], xt1_d[:, 2 * c0 : 4 * c0])
            nc.sync.dma_start(w1a_sbs[0][:, 4 * P :], w1a_d[0][:, 4 * P :])
            nc.gpsimd.dma_start(xt1_sb[:, 4 * c0 : 8 * c0], xt1_d[:, 4 * c0 : 8 * c0])
            nc.scalar.dma_start(b1a_sb[:], b1a_d[:])
            # w1 m=1..3 striped in halves across alternating queues
            nc.sync.dma_start(w1a_sbs[1][:, : 4 * P], w1a_d[1][:, : 4 * P])
            nc.scalar.dma_start(w1a_sbs[1][:, 4 * P :], w1a_d[1][:, 4 * P :])
            nc.gpsimd.dma_start(w1a_sbs[2][:, : 4 * P], w1a_d[2][:, : 4 * P])
            nc.sync.dma_start(w1a_sbs[2][:, 4 * P :], w1a_d[2][:, 4 * P :])
            nc.scalar.dma_start(w1a_sbs[3][:, : 4 * P], w1a_d[3][:, : 4 * P])
            nc.gpsimd.dma_start(w1a_sbs[3][:, 4 * P :], w1a_d[3][:, 4 * P :])
            # m=4..7 whole blocks round-robin
            nc.sync.dma_start(w1a_sbs[4][:], w1a_d[4])
            nc.scalar.dma_start(w1a_sbs[5][:], w1a_d[5])
            nc.gpsimd.dma_start(w1a_sbs[6][:], w1a_d[6])
            nc.sync.dma_start(w1a_sbs[7][:], w1a_d[7])
            if len(chunks1) > 1:
                nc.gpsimd.dma_start(xt1_sb[:, off1[1] :], xt1_d[:, off1[1] :])
            # rest of the w1a stream on sync
            for m in range(8, MH):
                nc.sync.dma_start(w1a_sbs[m][:], w1a_d[m])

            ht1_sb = ht_pool.tile([P, MH * C1], bf16)
            ht2_sb = ht_pool.tile([P, MH * C2], bf16)

            def l1_block(m, ci, off, t0, tn, w1_sb, C, xt_sb, ht_sb, b1_sb):
                ps = psum_pool.tile([P, 512], f32, tag=f"ps{ci}", name=f"ps{ci}")
                for k in range(KD):
                    nc.tensor.matmul(
                        ps[:, :tn],
                        lhsT=w1_sb[:, k * P : (k + 1) * P],
                        rhs=xt_sb[:, off + k * tn : off + (k + 1) * tn],
                        start=(k == 0),
                        stop=(k == KD - 1),
                    )
                nc.scalar.activation(
                    ht_sb[:, m * C + t0 : m * C + t0 + tn],
                    ps[:, :tn],
                    AF.Relu,
                    bias=b1_sb[:, m : m + 1],
                )

            # Layer 1, slot a: single-chunk passes (pass A only needs the
            # chunk-0 xt bytes; w1a tiles stay resident for pass B)
            t0 = 0
            for ci, tn in enumerate(chunks1):
                for m in range(MH):
                    l1_block(
                        m, 0, off1[ci], t0, tn, w1a_sbs[m], C1, xt1_sb, ht1_sb,
                        b1a_sb,
                    )
                if ci == 0:
                    # slot-b tokens/bias + first layer-2 weights prefetch on
                    # the now-idle gpsimd queue
                    nc.gpsimd.dma_start(xt2_sb[:], xt2_d[:])
                    nc.gpsimd.dma_start(b1b_sb[:], b1b_d[:])
                t0 += tn
            w2_first = w2_pool.tile([P, MH * P], bf16, tag="w2")
            H2 = MH * P // 2
            nc.gpsimd.dma_start(w2_first[:, :H2], w2b_d[0][:, :H2])
            nc.gpsimd.dma_start(w2_first[:, H2:], w2b_d[0][:, H2:])

            # Layer 1, slot b: paired chunks (k-outer), weights ring through
            # the w1 pool behind the resident w1a tiles
            for m in range(MH):
                w1_sb = w1_pool.tile([P, D], bf16, tag="w1")
                nc.sync.dma_start(w1_sb[:], w1b_d[m])
                pss = [
                    psum_pool.tile([P, 512], f32, tag=f"ps{ci}", name=f"ps{ci}")
                    for ci in range(len(chunks2))
                ]
                for k in range(KD):
                    for ci, tn in enumerate(chunks2):
                        nc.tensor.matmul(
                            pss[ci][:, :tn],
                            lhsT=w1_sb[:, k * P : (k + 1) * P],
                            rhs=xt2_sb[:, off2[ci] + k * tn : off2[ci] + (k + 1) * tn],
                            start=(k == 0),
                            stop=(k == KD - 1),
                        )
                t0 = 0
                for ci, tn in enumerate(chunks2):
                    nc.scalar.activation(
                        ht2_sb[:, m * C2 + t0 : m * C2 + t0 + tn],
                        pss[ci][:, :tn],
                        AF.Relu,
                        bias=b1b_sb[:, m : m + 1],
                    )
                    t0 += tn

            def layer2(m2, w2_sb, C, chunks, ht_sb, yt_d, last=False):
                if last:
                    # final block: single-chunk passes so the earlier chunks'
                    # activations+stores overlap the later chunks' matmuls;
                    # the very last chunk's store is split across the two
                    # idle DMA engines to shorten the drain
                    t0 = 0
                    for ci, tn in enumerate(chunks):
                        ps = psum_pool.tile([P, 512], f32, tag="ps0", name="ps")
                        for k2 in range(MH):
                            nc.tensor.matmul(
                                ps[:, :tn],
                                lhsT=w2_sb[:, k2 * P : (k2 + 1) * P],
                                rhs=ht_sb[:, k2 * C + t0 : t0 + k2 * C + tn],
                                start=(k2 == 0),
                                stop=(k2 == MH - 1),
                            )
                        yt_sb = y_pool.tile([P, 512], bf16, tag="yt")
                        nc.scalar.activation(yt_sb[:, :tn], ps[:, :tn], AF.Identity)
                        if ci == len(chunks) - 1:
                            nc.sync.dma_start(
                                yt_d[m2][:, t0 : t0 + tn], yt_sb[:, :tn]
                            )
                        else:
                            nc.gpsimd.dma_start(
                                yt_d[m2][:, t0 : t0 + tn], yt_sb[:, :tn]
                            )
                        t0 += tn
                    return
                pss = [
                    psum_pool.tile([P, 512], f32, tag=f"ps{ci}", name=f"ps{ci}")
                    for ci in range(len(chunks))
                ]
                for k2 in range(MH):
                    t0 = 0
                    for ci, tn in enumerate(chunks):
                        nc.tensor.matmul(
                            pss[ci][:, :tn],
                            lhsT=w2_sb[:, k2 * P : (k2 + 1) * P],
                            rhs=ht_sb[:, k2 * C + t0 : k2 * C + t0 + tn],
                            start=(k2 == 0),
                            stop=(k2 == MH - 1),
                        )
                        t0 += tn
                t0 = 0
                for ci, tn in enumerate(chunks):
                    yt_sb = y_pool.tile([P, 512], bf16, tag="yt")
                    # partial sum over this core's DFF half; b2 added on host
                    nc.scalar.activation(yt_sb[:, :tn], pss[ci][:, :tn], AF.Identity)
                    nc.gpsimd.dma_start(yt_d[m2][:, t0 : t0 + tn], yt_sb[:, :tn])
                    t0 += tn

            for m2 in range(KD):
                if m2 == 0:
                    w2_sb = w2_first
                else:
                    w2_sb = w2_pool.tile([P, MH * P], bf16, tag="w2")
                    nc.sync.dma_start(w2_sb[:], w2b_d[m2])
                layer2(m2, w2_sb, C2, chunks2, ht2_sb, yt2_d)
            for m2 in range(KD):
                w2_sb = w2_pool.tile([P, MH * P], bf16, tag="w2")
                nc.sync.dma_start(w2_sb[:], w2a_d[m2])
                layer2(m2, w2_sb, C1, chunks1, ht1_sb, yt1_d, last=(m2 == KD - 1))

    nc.compile()
    return nc


def _arrange_tokens(x_e, C):
    """[n, D] tokens -> chunk-major xt: for each token chunk [t0:t0+tn],
    a [P, KD*tn] block with [p, k*tn + c] = x_e[t0+c, k*128 + p]."""
    xe = np.zeros((C, D), np.float32)
    xe[: len(x_e)] = x_e
    blocks = []
    t0 = 0
    for tn in _chunk_sizes(C):
        blk = xe[t0 : t0 + tn].T.reshape(KD, P, tn).transpose(1, 0, 2)
        blocks.append(blk.reshape(P, KD * tn))
        t0 += tn
    return _bf16(np.concatenate(blocks, axis=1))


def _arrange_w1_half(w1_e, h):
    """w1 half: [D, 2048] -> [MH, P, D] with [m, p, k*128+j] = w1[k*128+p, off+m*128+j]."""
    half = w1_e[:, h * (MH * P) : (h + 1) * (MH * P)]
    return _bf16(half.reshape(KD, P, MH, P).transpose(2, 1, 0, 3).reshape(MH, P, D))


def _arrange_w2_half(w2_e, h):
    """w2 half: [2048, D] -> [KD, P, 2048] with [m2, p, k2*128+j] = w2[off+k2*128+p, m2*128+j]."""
    half = w2_e[h * (MH * P) : (h + 1) * (MH * P), :]
    return _bf16(half.reshape(MH, P, KD, P).transpose(2, 1, 0, 3).reshape(KD, P, MH * P))


def kernel(x, gate_w, gate_b, w1, b1, w2, b2):
    global LAST_RESULT

    x = np.ascontiguousarray(np.asarray(x, dtype=np.float32))
    gate_w = np.asarray(gate_w, dtype=np.float32)
    gate_b = np.asarray(gate_b, dtype=np.float32)
    w1 = np.asarray(w1, dtype=np.float32)
    b1 = np.asarray(b1, dtype=np.float32)
    w2 = np.asarray(w2, dtype=np.float32)
    b2 = np.asarray(b2, dtype=np.float32)
    n_tok = x.shape[0]

    # host gate + top-1 routing (fp64: exact argmax, see module docstring)
    logits = x.astype(np.float64) @ gate_w.astype(np.float64) + gate_b.astype(
        np.float64
    )
    assign = np.argmax(logits, axis=-1)
    idx_full = [np.nonzero(assign == e)[0] for e in range(E)]

    # Defensive slabbing: if routing were pathologically imbalanced, process
    # tokens in passes so per-expert capacity stays within SBUF limits. With
    # the benchmark's near-uniform gate this is a single pass.
    slab = 960
    n_pass = max(1, -(-max(len(i) for i in idx_full) // slab))
    out = np.zeros((n_tok, D), np.float32)
    for ps in range(n_pass):
        idx = [i[ps * slab : (ps + 1) * slab] for i in idx_full]
        _run_pass(x, w1, b1, w2, b2, idx, out)
    return out


def _run_pass(x, w1, b1, w2, b2, idx, out):
    from concourse.bass_utils import run_bass_kernel_spmd

    global LAST_RESULT

    counts = np.array([len(i) for i in idx])

    # pair experts large-with-small to balance per-core token load
    order = np.argsort(-counts, kind="stable")
    pairs = [(int(order[p]), int(order[E - 1 - p])) for p in range(E // 2)]
    C1 = _pad_cap(max(counts[a] for a, _ in pairs))
    C2 = _pad_cap(max(counts[b] for _, b in pairs))

    key = (C1, C2)
    if key not in _PROGRAM_CACHE:
        _PROGRAM_CACHE[key] = _build_program_tp2(C1, C2)
    nc = _PROGRAM_CACHE[key]

    in_maps = []
    for c in range(E):
        p, h = divmod(c, 2)
        ea, eb = pairs[p]
        in_maps.append(
            {
                "xt1": _arrange_tokens(x[idx[ea]], C1),
                "xt2": _arrange_tokens(x[idx[eb]], C2),
                "w1a": _arrange_w1_half(w1[ea], h),
                "w1b": _arrange_w1_half(w1[eb], h),
                "b1a": np.ascontiguousarray(
                    b1[ea][h * (MH * P) : (h + 1) * (MH * P)].reshape(MH, P).T
                ),
                "b1b": np.ascontiguousarray(
                    b1[eb][h * (MH * P) : (h + 1) * (MH * P)].reshape(MH, P).T
                ),
                "w2a": _arrange_w2_half(w2[ea], h),
                "w2b": _arrange_w2_half(w2[eb], h),
            }
        )

    res = run_bass_kernel_spmd(
        nc,
        in_maps,
        core_ids=list(range(E)),
        trace=TRACE,
        **({"trace_cores": TRACE_CORES} if TRACE_CORES else {}),
    )
    LAST_RESULT = res

    for p in range(E // 2):
        ea, eb = pairs[p]
        for slot, e in (("yt1", ea), ("yt2", eb)):
            n = len(idx[e])
            if n == 0:
                continue
            # sum the two DFF-half partials, restore [tokens, D], add b2
            yt = res.results[2 * p][slot].astype(np.float32) + res.results[
                2 * p + 1
            ][slot].astype(np.float32)
            ye = yt.transpose(2, 0, 1).reshape(-1, D)
            out[idx[e]] = ye[:n] + b2[e]
